# revision 10
# baseline (speedup 1.0000x reference)
"""DiffusionStep kernel v3: one-hot matmul scatter, no GPSIMD scatter_add.

out[src] += w * x[dst], N=100000 nodes, E=6400000 edges, d=1.

Per core (EC=800768 padded edges): node n = a*128 + p (p = n & 127,
a = n >> 7, a < 784). Edges processed in 128-row column tiles:

  gather:  ap_gather 16-residue candidates from xtab [128, 6250] fp16,
           mask by (dst & 15 == partition % 16), 16->1 block-ones matmul
           writes m = x[dst] straight into PSUM m-tile rows [8k, 8k+8).
  scatter: per column c (128 edges): DVE tensor_scalar builds
           oh  = (iotaA == srca_c)            [128, 784] fp16  (2x mode)
           ml  = (iotaP == srcp_c) * mw_c     [128, 128] fp16  (2x mode)
           two PE matmuls accumulate acc[p, a] += ml.T @ oh into
           PSUM accA [128,512] / accB [128,272] f32.
  combine: AllReduce partial [128, 784] over 8 cores, relayout to out.
"""
import sys

sys.path.insert(0, "/opt/trn_rl_repo")

import numpy as np

N = 100000
E = 6400000
NCORES = 8
EC = E // NCORES          # 800000
SBS = [2048, 2048, 2048, 112]   # columns per superblock (128 edges/col)
NCOL = sum(SBS)           # 6256
ECP = NCOL * 128          # 800768 (768 pad edges, w=0)
NA = 784                  # a = src >> 7 in [0, 782); padded to 784
NT = 6250                 # xtab elems per row = N / 16

_cache = {}


def _build(passes=1):
    from concourse import bacc, mybir, tile

    nc = bacc.Bacc(None, target_bir_lowering=False)
    f32, f16, i32, i16 = (
        mybir.dt.float32, mybir.dt.float16, mybir.dt.int32, mybir.dt.int16,
    )
    EQ = mybir.AluOpType.is_equal
    MUL = mybir.AluOpType.mult

    dstw = nc.declare_dram_parameter("dstw", [128, NCOL], i16, isOutput=False)
    dstr = nc.declare_dram_parameter("dstr", [128, 16 * NCOL], f32, isOutput=False)
    srcv = nc.declare_dram_parameter("srcv", [128, NCOL], i32, isOutput=False)
    wv = nc.declare_dram_parameter("wv", [128, NCOL], f16, isOutput=False)
    xtab = nc.declare_dram_parameter("xtab", [128, NT], f32, isOutput=False)
    iotaA = nc.declare_dram_parameter("iotaA", [128, NA], f16, isOutput=False)
    iotaP = nc.declare_dram_parameter("iotaP", [128, 128], f16, isOutput=False)
    iotares = nc.declare_dram_parameter("iotares", [128, 1], f32, isOutput=False)
    bsh = nc.declare_dram_parameter("bsh", [128, 2048], f32, isOutput=False)
    out = nc.declare_dram_parameter("out", [N, 1], f32, isOutput=True)

    partial = nc.dram_tensor("partial", [128, NA], f32)
    reduced = nc.dram_tensor("reduced", [128, NA], f32, addr_space="Shared")

    with tile.TileContext(nc) as tc:
        with tc.tile_pool(name="const", bufs=1) as cpool, \
             tc.tile_pool(name="gw", bufs=2) as gpool, \
             tc.tile_pool(name="sb", bufs=2) as spool, \
             tc.tile_pool(name="sc", bufs=1) as scpool, \
             tc.tile_pool(name="oh", bufs=4) as ohpool, \
             tc.tile_pool(name="ml", bufs=4) as mlpool, \
             tc.tile_pool(name="mp", bufs=1, space="PSUM") as mppool, \
             tc.tile_pool(name="ac", bufs=1, space="PSUM") as apool:

            xtab_sb = cpool.tile([128, NT], f32, tag="xtab")
            nc.sync.dma_start(out=xtab_sb[:], in_=xtab[:])
            iotaA_sb = cpool.tile([128, NA], f16, tag="iotaA")
            nc.sync.dma_start(out=iotaA_sb[:], in_=iotaA[:])
            iotaP_sb = cpool.tile([128, 128], f16, tag="iotaP")
            nc.sync.dma_start(out=iotaP_sb[:], in_=iotaP[:])
            iores_sb = cpool.tile([128, 1], f32, tag="iores")
            nc.sync.dma_start(out=iores_sb[:], in_=iotares[:])
            bsh_sb = cpool.tile([128, 2048], f32, tag="bsh")
            nc.sync.dma_start(out=bsh_sb[:], in_=bsh[:])

            accA = apool.tile([128, 512], f32, tag="accA")
            accB = apool.tile([128, 272], f32, tag="accB")

            ntile = passes * NCOL
            it = 0
            for pss in range(passes):
                coloff = 0
                for CH in SBS:
                    qc = CH // 16
                    mps = mppool.tile([128, 2048], f32, tag="mps")
                    for k in range(16):
                        dw = gpool.tile([128, qc], i16, tag="dw")
                        nc.sync.dma_start(
                            out=dw[:],
                            in_=dstw[:, coloff + k * qc: coloff + (k + 1) * qc])
                        dr = gpool.tile([128, CH], f32, tag="dr")
                        b = 16 * coloff + k * CH
                        nc.sync.dma_start(out=dr[:], in_=dstr[:, b: b + CH])
                        cand = gpool.tile([128, CH], f32, tag="cand")
                        nc.gpsimd.ap_gather(
                            out_ap=cand[:], in_ap=xtab_sb[:], idxs_ap=dw[:],
                            channels=128, num_elems=NT, d=1, num_idxs=CH,
                        )
                        nc.vector.tensor_scalar(
                            out=dr[:], in0=dr[:], scalar1=iores_sb[:, 0:1],
                            scalar2=None, op0=EQ,
                        )
                        nc.vector.tensor_tensor(
                            out=cand[:], in0=cand[:], in1=dr[:], op=MUL)
                        for s in range(0, CH, 512):
                            e = min(CH, s + 512)
                            nc.tensor.matmul(
                                out=mps[:, s:e],
                                lhsT=bsh_sb[:, 128 * k: 128 * (k + 1)],
                                rhs=cand[:, s:e],
                                start=(k == 0), stop=(k == 15),
                            )
                    src_c = scpool.tile([128, CH], i32, tag="srcc")
                    nc.sync.dma_start(out=src_c[:],
                                      in_=srcv[:, coloff: coloff + CH])
                    w_c = spool.tile([128, CH], f16, tag="wc")
                    nc.sync.dma_start(out=w_c[:],
                                      in_=wv[:, coloff: coloff + CH])
                    srca = scpool.tile([128, CH], i32, tag="srca")
                    nc.vector.tensor_scalar(
                        out=srca[:], in0=src_c[:], scalar1=7, scalar2=None,
                        op0=mybir.AluOpType.logical_shift_right)
                    srca32 = spool.tile([128, CH], f32, tag="srca32")
                    nc.vector.tensor_copy(out=srca32[:], in_=srca[:])
                    nc.vector.tensor_scalar(
                        out=src_c[:], in0=src_c[:], scalar1=127, scalar2=None,
                        op0=mybir.AluOpType.bitwise_and)
                    srcp32 = spool.tile([128, CH], f32, tag="srcp32")
                    nc.vector.tensor_copy(out=srcp32[:], in_=src_c[:])
                    m_sb = spool.tile([128, CH], f16, tag="msb")
                    nc.scalar.copy(out=m_sb[:], in_=mps[:, 0:CH])
                    mw = spool.tile([128, CH], f32, tag="mw")
                    nc.vector.tensor_tensor(
                        out=mw[:], in0=m_sb[:], in1=w_c[:], op=MUL)
                    for c in range(CH):
                        oh = ohpool.tile([128, NA], f16, tag="oh")
                        nc.vector.tensor_scalar(
                            out=oh[:], in0=iotaA_sb[:],
                            scalar1=srca32[:, c:c + 1], scalar2=None, op0=EQ)
                        ml = mlpool.tile([128, 128], f16, tag="ml")
                        nc.vector.tensor_scalar(
                            out=ml[:], in0=iotaP_sb[:],
                            scalar1=srcp32[:, c:c + 1],
                            scalar2=mw[:, c:c + 1], op0=EQ, op1=MUL)
                        first = it == 0
                        last = it == ntile - 1
                        nc.tensor.matmul(
                            out=accA[:], lhsT=ml[:], rhs=oh[:, 0:512],
                            start=first, stop=last)
                        nc.tensor.matmul(
                            out=accB[:], lhsT=ml[:], rhs=oh[:, 512:NA],
                            start=first, stop=last)
                        it += 1
                    coloff += CH

            part_sb = cpool.tile([128, NA], f32, tag="part")
            nc.vector.tensor_copy(out=part_sb[:, 0:512], in_=accA[:])
            nc.vector.tensor_copy(out=part_sb[:, 512:NA], in_=accB[:])
            nc.sync.dma_start(out=partial[:], in_=part_sb[:])
            nc.gpsimd.collective_compute(
                "AllReduce",
                mybir.AluOpType.add,
                replica_groups=[list(range(NCORES))],
                ins=[partial[:]],
                outs=[reduced[:]],
            )
            nmain = (N // 128) * 128  # 99968
            with nc.allow_non_contiguous_dma(reason="final relayout"):
                for a0, a1 in ((0, 448), (448, nmain // 128)):
                    nc.sync.dma_start(
                        out=out[a0 * 128: a1 * 128, 0]
                            .rearrange("(a p) -> a p", p=128),
                        in_=reduced[:, a0: a1].rearrange("p a -> a p"),
                    )
                nc.sync.dma_start(
                    out=out[nmain:N, 0:1],
                    in_=reduced[0: N - nmain, nmain // 128: nmain // 128 + 1],
                )

    nc.finalize()
    return nc


def _get_nc():
    if "nc1" not in _cache:
        _cache["nc1"] = _build(passes=1)
    return _cache["nc1"]


def _build_timing(passes=2):
    key = ("t", passes)
    if key not in _cache:
        _cache[key] = _build(passes=passes)
    return _cache[key]


def _host_prep(x, edge_index, edge_weight, *_unused):
    x = np.asarray(x, dtype=np.float32).reshape(N)
    ei = np.asarray(edge_index)
    src = ei[0].astype(np.int32)
    dst = ei[1].astype(np.int32)
    w = np.asarray(edge_weight, dtype=np.float32)

    xtab = np.tile(np.ascontiguousarray(x.reshape(NT, 16).T), (8, 1)) \
        .astype(np.float32)
    iotaA = np.tile(np.arange(NA, dtype=np.float16), (128, 1))
    iotaP = np.tile(np.arange(128, dtype=np.float16), (128, 1))
    iotares = (np.arange(128) % 16).astype(np.float32).reshape(128, 1)
    bsh = np.zeros((128, 2048), np.float32)
    for k in range(16):
        for g in range(8):
            bsh[16 * g:16 * (g + 1), 128 * k + 8 * k + g] = 1.0

    in_maps = []
    for ci in range(NCORES):
        lo = ci * EC
        s = np.zeros(ECP, np.int32)
        d = np.zeros(ECP, np.int32)
        ww = np.zeros(ECP, np.float32)
        s[:EC] = src[lo:lo + EC]
        d[:EC] = dst[lo:lo + EC]
        ww[:EC] = w[lo:lo + EC]

        dstw_l, dstr_l, src_l, w_l = [], [], [], []
        off = 0
        for CH in SBS:
            ne = 128 * CH
            s3 = s[off:off + ne].reshape(16, 8, CH)
            d3 = d[off:off + ne].reshape(16, 8, CH)
            w3 = ww[off:off + ne].reshape(16, 8, CH)
            off += ne
            dw = (d3 >> 4).astype(np.int16).reshape(16, 8, CH // 16, 16)
            dw = dw.transpose(1, 3, 0, 2).reshape(128, CH)
            dstw_l.append(dw)
            dr = (d3 & 15).astype(np.float32).transpose(1, 0, 2)  # [g,k,c]
            dr = np.repeat(dr[:, None, :, :], 16, axis=1)         # [g,r,k,c]
            dstr_l.append(dr.reshape(128, 16 * CH))
            src_l.append(s3.reshape(128, CH))
            w_l.append(w3.astype(np.float16).reshape(128, CH))

        in_maps.append({
            "dstw": np.ascontiguousarray(np.concatenate(dstw_l, 1)),
            "dstr": np.ascontiguousarray(np.concatenate(dstr_l, 1)),
            "srcv": np.ascontiguousarray(np.concatenate(src_l, 1)),
            "wv": np.ascontiguousarray(np.concatenate(w_l, 1)),
            "xtab": xtab,
            "iotaA": iotaA,
            "iotaP": iotaP,
            "iotares": iotares,
            "bsh": bsh,
        })
    return in_maps


def kernel(x, edge_index, edge_weight):
    from concourse.bass_utils import run_bass_kernel_spmd

    nc = _get_nc()
    in_maps = _host_prep(x, edge_index, edge_weight)
    res = run_bass_kernel_spmd(nc, in_maps, list(range(NCORES)))
    out = res.results[0]["out"].astype(np.float32).reshape(N, 1)
    return out


NCH_FULL = None  # kept for test.py compatibility
